# revision 21
# baseline (speedup 1.0000x reference)
"""MultiHeadLatentAttention Trainium2 kernel (8 NeuronCores, SPMD).

Sharding: core c -> (batch b = c // 4, latent group g = c % 4).
Each core owns query heads [4g, 4g+4) and latent head g for its batch:
  - q projection for its 4 heads (1/sqrt(HD) folded into the weights)
  - k, v via HOST-MERGED latent weights: k = x @ (kr_w@kl_w)^T + (kr_w@kl_b
    + kr_b) -- mathematically identical to the two-stage latent form but
    4.25x fewer FLOPs since head_dim(128) << latent_dim(512)
  - causal attention for 4 heads in transposed layout (scoresT[s_k, s_q]):
    exp on ScalarE straight out of PSUM (no max subtraction -- scores are
    O(1) by construction), structural causality (diagonal blocks masked by
    a 0/1 tril multiply on probs; score/AV matmuls and prob handling are
    sliced to skip fully-masked column ranges), softmax denominators via
    bf16 accumulation of prob tiles on the DVE (two parity chains per unit)
    followed by a single all-ones [128x128] stationary matmul per unit,
    normalization by DVE fast-reciprocal multiply
  - partial o_proj (its 512 input dims -> full 2048 output dims), bf16 out
Host sums the 4 partial o_proj outputs per batch (fp32) and adds o_b.

The loop is software-pipelined one chunk deep: chunk n's projections (pure
PE work) are emitted interleaved, matmul by matmul, into chunk n-1's
attention stream, so the PE stays busy while the ScalarE exp chain (the
attention-phase rate limiter at ~600ns/block vs the PE's ~430ns/block)
catches up. kT/v are double-buffered by repeat-body parity so the last
chunk's attention overlaps the next body's first projections; attention for
the last chunk of the final body drains unoverlapped. x chunks prefetch one
chunk ahead on the SP/gpsimd DMA queues, ordered ahead of the output
stores so stores never block the prefetch stream.

All matmuls run in bf16 with fp32 PSUM accumulation; each attention unit's
tail (deferred AVs, denominator matmul, normalize) is deferred past the
next unit's score matmuls so the PE never waits on the exp chain.
"""

import math

import numpy as np
import ml_dtypes

B, S, H = 2, 2048, 2048
NH, HD = 16, 128
NLH, LD = 4, 512
HPC = 4            # query heads per core
NCORES = 8
SQC = 512          # s_q chunk
NQC = S // SQC     # 4 chunks
NKT = H // 128     # 16 contraction tiles for the projections
NSB = S // 128     # 16 s_k blocks
BF16 = ml_dtypes.bfloat16

_CACHE = {}


def _build_program(repeat=1):
    import concourse.bacc as bacc
    import concourse.bass as bass
    import concourse.tile as tile
    from concourse import mybir
    from contextlib import ExitStack

    dt = mybir.dt
    AF = mybir.ActivationFunctionType

    nc = bacc.Bacc("TRN2", target_bir_lowering=False, debug=False,
                   num_devices=NCORES)

    xT = nc.declare_dram_parameter("xT", [H, S], dt.bfloat16, isOutput=False)
    qw = nc.declare_dram_parameter("qwT", [H, HPC * HD], dt.bfloat16, isOutput=False)
    kw = nc.declare_dram_parameter("kwT", [H, HD], dt.bfloat16, isOutput=False)
    vw = nc.declare_dram_parameter("vwT", [H, HD], dt.bfloat16, isOutput=False)
    ow = nc.declare_dram_parameter("owT", [HPC * HD, H], dt.bfloat16, isOutput=False)
    qb = nc.declare_dram_parameter("qb", [HPC * HD], dt.float32, isOutput=False)
    kb = nc.declare_dram_parameter("kb", [HD], dt.float32, isOutput=False)
    vb = nc.declare_dram_parameter("vb", [HD], dt.float32, isOutput=False)
    tri = nc.declare_dram_parameter("tri", [128, 128], dt.bfloat16, isOutput=False)
    outp = nc.declare_dram_parameter("out", [H, S], dt.bfloat16, isOutput=True)

    with tile.TileContext(nc) as tc, ExitStack() as ctx:
        const = ctx.enter_context(tc.tile_pool(name="const", bufs=1))
        xpool = ctx.enter_context(tc.tile_pool(name="xpool", bufs=16))
        probs_pool = ctx.enter_context(tc.tile_pool(name="probs", bufs=16))
        attn_pool = ctx.enter_context(tc.tile_pool(name="attn", bufs=8))
        small = ctx.enter_context(tc.tile_pool(name="small", bufs=8))
        accp = ctx.enter_context(tc.tile_pool(name="accp", bufs=6))
        psum = ctx.enter_context(tc.tile_pool(name="psum", bufs=8, space="PSUM"))

        # ---------------- constants / weights ----------------
        # first x chunk (gpsimd queues) + q weights (sync queues) land first so
        # the PE can start ASAP; k-interleaved so (qw[k], xt[k]) pairs arrive
        # in consumption order.
        qw_sb = const.tile([128, NKT, HPC * HD], dt.bfloat16, tag="qw")
        kw_sb = const.tile([128, NKT, HD], dt.bfloat16, tag="kw")
        vw_sb = const.tile([128, NKT, HD], dt.bfloat16, tag="vw")
        for k4 in range(4):
            nc.sync.dma_start(
                out=kw_sb[:, 4 * k4:4 * (k4 + 1), :],
                in_=kw.ap()[512 * k4:512 * (k4 + 1), :]
                .rearrange("(k p) m -> p k m", p=128))
        for k4 in range(4):
            nc.sync.dma_start(
                out=vw_sb[:, 4 * k4:4 * (k4 + 1), :],
                in_=vw.ap()[512 * k4:512 * (k4 + 1), :]
                .rearrange("(k p) m -> p k m", p=128))
        xs0 = []
        for g in range(4):
            xt = xpool.tile([128, 4, SQC], dt.bfloat16, tag="xt", name="xt")
            nc.gpsimd.dma_start(
                out=xt,
                in_=xT.ap()[512 * g:512 * (g + 1), 0:SQC]
                .rearrange("(k p) m -> p k m", p=128))
            xs0.append(xt)
        for k in range(NKT):
            eng = nc.sync if k < 6 else nc.gpsimd
            eng.dma_start(out=qw_sb[:, k, :], in_=qw.ap()[128 * k:128 * (k + 1), :])

        qb_sb = const.tile([128, HPC], dt.float32, tag="qb")
        nc.sync.dma_start(out=qb_sb, in_=qb.ap().rearrange("(m p) -> p m", p=128))
        kb_sb = const.tile([128, 1], dt.float32, tag="kb")
        nc.sync.dma_start(out=kb_sb, in_=kb.ap().rearrange("(m p) -> p m", p=128))

        vb_ap = vb.ap()
        vb_bc = const.tile([128, HD], dt.float32, tag="vbb")
        nc.sync.dma_start(
            out=vb_bc,
            in_=bass.AP(tensor=vb_ap.tensor, offset=vb_ap.offset,
                        ap=[[0, 128]] + list(vb_ap.ap)),
        )
        tri_sb = const.tile([128, 128], dt.bfloat16, tag="tri")
        nc.sync.dma_start(out=tri_sb, in_=tri.ap())
        ones_sb = const.tile([128, 128], dt.bfloat16, tag="ones")
        nc.vector.memset(ones_sb, 1.0)

        # o_proj weights: first consumed ~45us in (o_proj of iq=0); loaded
        # once in the preamble, after the startup-critical tiles.
        ow_sb = const.tile([128, 4, H], dt.bfloat16, tag="ow")
        nc.sync.dma_start(
            out=ow_sb, in_=ow.ap().rearrange("(k p) m -> p k m", p=128))

        # persistent activations; kT/v double-buffered by repeat-body parity
        # so the last chunk's attention can overlap the next body's first
        # projections (which overwrite chunk 0).
        qT_sb = [const.tile([128, S], dt.bfloat16, tag=f"qT{h}", name=f"qT{h}")
                 for h in range(HPC)]
        kT_sb = [const.tile([128, S], dt.bfloat16, tag=f"kT{p}", name=f"kT{p}")
                 for p in range(2)]
        v_sb = [const.tile([128, NSB, HD], dt.bfloat16, tag=f"v{p}", name=f"v{p}")
                for p in range(2)]

        def _load_chunk(n):
            # 4 k-slices bundled per tile/DMA: fewer descriptors, and the
            # 4 matmuls sharing a tile coalesce their semaphore waits.
            xs = []
            for g in range(4):
                xt = xpool.tile([128, 4, SQC], dt.bfloat16, tag="xt", name="xt")
                eng = nc.sync if g % 2 == 0 else nc.gpsimd
                eng.dma_start(
                    out=xt,
                    in_=xT.ap()[512 * g:512 * (g + 1), SQC * n:SQC * (n + 1)]
                    .rearrange("(k p) m -> p k m", p=128))
                xs.append(xt)
            return xs

        def _proj_closures(n, parity, xs):
            """Chunk-n projection emission, split into 4-matmul closures (one
            accumulation sub-chain each, preserving semaphore coalescing) so
            attention emission can splice them in as PE filler."""
            kTd, vd = kT_sb[parity], v_sb[parity]
            cls = []
            box = {}

            def kt(k4):
                def go():
                    if k4 == 0:
                        box["kt"] = psum.tile([128, SQC], dt.float32, tag="bank",
                                              name="ps_kt")
                    for k in range(4 * k4, 4 * k4 + 4):
                        nc.tensor.matmul(box["kt"], lhsT=kw_sb[:, k, :],
                                         rhs=xs[k // 4][:, k % 4, :],
                                         start=(k == 0), stop=(k == NKT - 1))
                    if k4 == 3:
                        nc.scalar.activation(
                            out=kTd[:, SQC * n:SQC * (n + 1)], in_=box["kt"],
                            func=AF.Identity, bias=kb_sb[:, 0:1])
                return go

            cls += [kt(k4) for k4 in range(4)]

            def vq(jj, k4):
                def go():
                    if k4 == 0:
                        box[f"v{jj}"] = psum.tile([128, SQC], dt.float32,
                                                  tag="bank", name="ps_v")
                    ps = box[f"v{jj}"]
                    for k in range(4 * k4, 4 * k4 + 4):
                        nc.tensor.matmul(
                            ps[:, :HD],
                            lhsT=xs[k // 4][:, k % 4, 128 * jj:128 * (jj + 1)],
                            rhs=vw_sb[:, k, :],
                            start=(k == 0), stop=(k == NKT - 1))
                    if k4 == 3:
                        nc.vector.tensor_add(out=vd[:, 4 * n + jj, :],
                                             in0=ps[:, :HD], in1=vb_bc)
                return go

            cls += [vq(jj, k4) for jj in range(4) for k4 in range(4)]

            def qh(h, k4):
                def go():
                    if k4 == 0:
                        box[f"q{h}"] = psum.tile([128, SQC], dt.float32,
                                                 tag="bank", name="ps_q")
                    for k in range(4 * k4, 4 * k4 + 4):
                        nc.tensor.matmul(box[f"q{h}"],
                                         lhsT=qw_sb[:, k, 128 * h:128 * (h + 1)],
                                         rhs=xs[k // 4][:, k % 4, :],
                                         start=(k == 0),
                                         stop=(k == NKT - 1))
                    if k4 == 3:
                        nc.scalar.activation(
                            out=qT_sb[h][:, SQC * n:SQC * (n + 1)],
                            in_=box[f"q{h}"], func=AF.Identity,
                            bias=qb_sb[:, h:h + 1])
                return go

            cls += [qh(h, k4) for h in range(HPC) for k4 in range(4)]
            return cls

        def _emit_attn(iq, parity, fill):
            """Attention for s_q chunk iq against kT/v buffer `parity`.
            Each unit's tail is deferred past the next unit's scores; `fill`
            emits pending projection matmuls as PE filler."""
            kTd, vd = kT_sb[parity], v_sb[parity]
            attn_tiles = []
            prev_tail = None

            def make_unit(h):
                J = 4 * iq + 4
                state = {"av": None}
                accs = [None, None]
                pending = []

                def emit_av(j, pt, lo):
                    if state["av"] is None:
                        state["av"] = psum.tile([128, SQC], dt.float32,
                                                tag="bank", name="ps_av")
                    nc.tensor.matmul(state["av"][:, lo:], lhsT=vd[:, j, :],
                                     rhs=pt[:, lo:],
                                     start=(j == 0), stop=(j == J - 1),
                                     skip_group_check=True)

                def emit_scores():
                    for j in range(J):
                        d = j - 4 * iq
                        lo = 128 * d if d > 0 else 0
                        ps_s = psum.tile([128, SQC], dt.float32, tag="bank",
                                         name="ps_s")
                        nc.tensor.matmul(
                            ps_s[:, lo:],
                            lhsT=kTd[:, 128 * j:128 * (j + 1)],
                            rhs=qT_sb[h][:, SQC * iq + lo:SQC * (iq + 1)],
                            start=True, stop=True)
                        pt = probs_pool.tile([128, SQC], dt.bfloat16, tag="pt",
                                             name="pt")
                        if lo > 0:
                            # zero the fully-masked region so the accumulation
                            # chain can run at full extent; the AV matmul
                            # still skips it. (On DVE: gpsimd memsets would
                            # block SWDGE DMA dispatch behind them.)
                            nc.vector.memset(pt[:, :lo], 0.0)
                        nc.scalar.activation(out=pt[:, lo:], in_=ps_s[:, lo:],
                                             func=AF.Exp)
                        if d >= 0:
                            nc.vector.tensor_mul(
                                out=pt[:, 128 * d:128 * (d + 1)],
                                in0=pt[:, 128 * d:128 * (d + 1)], in1=tri_sb)
                        # parity-split accumulation chains, ping-ponged so no
                        # DVE op writes its own input (in-place RMW stalls the
                        # DVE pipeline on hardware)
                        c = j % 2
                        if accs[c] is None:
                            accs[c] = pt
                        else:
                            nxt = accp.tile([128, SQC], dt.bfloat16,
                                            tag="acc", name="acc")
                            nc.vector.tensor_add(out=nxt, in0=accs[c], in1=pt)
                            accs[c] = nxt
                        if j % 2 == 1:
                            fill(1)
                        pending.append((j, pt, lo))
                        if len(pending) > 3:
                            emit_av(*pending.pop(0))

                def emit_tail():
                    for p in pending:
                        emit_av(*p)
                        fill(1)
                    comb = accp.tile([128, SQC], dt.bfloat16, tag="acc",
                                     name="comb")
                    nc.vector.tensor_add(out=comb, in0=accs[0], in1=accs[1])
                    den = psum.tile([128, SQC], dt.float32, tag="bank",
                                    name="ps_den")
                    nc.tensor.matmul(den, lhsT=ones_sb, rhs=comb,
                                     start=True, stop=True)
                    recip = small.tile([128, SQC], dt.float32, tag="recip",
                                       name="recip")
                    nc.vector.reciprocal_approx_fast(out=recip, in_=den)
                    at = attn_pool.tile([128, SQC], dt.bfloat16, tag="at",
                                        name="at")
                    nc.vector.tensor_mul(out=at, in0=state["av"], in1=recip)
                    attn_tiles.append(at)

                return emit_scores, emit_tail

            for h in range(HPC):
                emit_scores, emit_tail = make_unit(h)
                emit_scores()
                if prev_tail is not None:
                    prev_tail()
                prev_tail = emit_tail
            prev_tail()
            return attn_tiles

        def _emit_oproj(iq, attn_tiles, fill):
            for m in range(NSB):
                fill(1)
                ps_o = psum.tile([128, SQC], dt.float32, tag="bank")
                for h in range(HPC):
                    nc.tensor.matmul(ps_o, lhsT=ow_sb[:, h, 128 * m:128 * (m + 1)],
                                     rhs=attn_tiles[h], start=(h == 0), stop=(h == 3))
                # PSUM unload split across ScalarE (idle during o_proj) and
                # DVE so neither becomes the phase bottleneck.
                o_sb = small.tile([128, SQC], dt.bfloat16, tag="osb")
                if m % 2 == 0:
                    nc.scalar.activation(out=o_sb, in_=ps_o, func=AF.Identity)
                else:
                    nc.vector.tensor_copy(out=o_sb, in_=ps_o)
                nc.sync.dma_start(
                    out=outp.ap()[128 * m:128 * (m + 1), SQC * iq:SQC * (iq + 1)],
                    in_=o_sb)

        def _nofill(k=1):
            pass

        # ---------------- software-pipelined emission ----------------
        # x is prefetched a whole body ahead: body r's last phase issues all
        # of body r+1's chunk loads (the 64-buffer rotation makes chunk c's
        # DMA wait for body r's chunk-c consumers automatically), so filler
        # projections never wait on in-flight DMA mid-attention.
        xs_body = [xs0] + [_load_chunk(c) for c in range(1, NQC)]
        xs_next = None
        carry = None  # (iq=3, parity) attention left over from the prior body
        for rep in range(repeat):
            parity = rep % 2
            for n in range(NQC):
                xs = xs_body[n]
                fillers = _proj_closures(n, parity, xs)
                fi = [0]

                def fill(k=1, fillers=fillers, fi=fi):
                    while k > 0 and fi[0] < len(fillers):
                        fillers[fi[0]]()
                        fi[0] += 1
                        k -= 1

                if n == NQC - 1 and rep < repeat - 1:
                    xs_next = [_load_chunk(c) for c in range(NQC)]

                prev = carry if n == 0 else (n - 1, parity)
                carry = None
                if prev is not None:
                    at = _emit_attn(prev[0], prev[1], fill)
                    _emit_oproj(prev[0], at, fill)
                fill(1 << 30)
            carry = (3, parity)
            xs_body = xs_next

        # drain: final body's last-chunk attention, unoverlapped
        at = _emit_attn(carry[0], carry[1], _nofill)
        _emit_oproj(carry[0], at, _nofill)

    nc.compile()
    return nc


def _get_nc(repeat=1):
    key = f"nc{repeat}"
    if key not in _CACHE:
        _CACHE[key] = _build_program(repeat)
    return _CACHE[key]


def _make_in_maps(hidden_states, attention_mask, q_w, q_b, kl_w, kl_b, vl_w, vl_b,
                  kr_w, kr_b, vr_w, vr_b, o_w):
    scale = 1.0 / math.sqrt(HD)
    tri01 = (np.asarray(attention_mask[0, 0, :128, :128]) == 0).T.astype(BF16)
    kr_f = np.asarray(kr_w, np.float32)
    vr_f = np.asarray(vr_w, np.float32)
    in_maps = []
    for c in range(NCORES):
        b, g = divmod(c, NLH)
        sl = slice(LD * g, LD * (g + 1))
        xTc = np.ascontiguousarray(np.asarray(hidden_states[b], np.float32).T
                                   ).astype(BF16)
        # merged latent->head weights: k = x @ (kr_w @ kl_w)^T + (kr_w@kl_b + kr_b)
        kw_eff = kr_f @ np.asarray(kl_w[sl], np.float32)
        vw_eff = vr_f @ np.asarray(vl_w[sl], np.float32)
        kb_eff = kr_f @ np.asarray(kl_b[sl], np.float32) + np.asarray(kr_b, np.float32)
        vb_eff = vr_f @ np.asarray(vl_b[sl], np.float32) + np.asarray(vr_b, np.float32)
        in_maps.append({
            "xT": xTc,
            "qwT": np.ascontiguousarray(
                (np.asarray(q_w[sl], np.float32) * scale).T).astype(BF16),
            "kwT": np.ascontiguousarray(kw_eff.T).astype(BF16),
            "vwT": np.ascontiguousarray(vw_eff.T).astype(BF16),
            "owT": np.ascontiguousarray(np.asarray(o_w, np.float32)[:, sl].T
                                        ).astype(BF16),
            "qb": (np.asarray(q_b[sl], np.float32) * scale),
            "kb": kb_eff,
            "vb": vb_eff,
            "tri": tri01,
        })
    return in_maps


def _gather(results, o_b):
    o_b = np.asarray(o_b, np.float32)
    outs = []
    for b in range(B):
        acc = np.zeros((H, S), np.float32)
        for g in range(NLH):
            acc += results[b * NLH + g]["out"].astype(np.float32)
        outs.append(acc.T + o_b[None, :])
    return np.stack(outs).astype(np.float32)


def kernel(hidden_states, position_ids, attention_mask, q_w, q_b, kl_w, kl_b,
           vl_w, vl_b, kr_w, kr_b, vr_w, vr_b, o_w, o_b):
    from concourse.bass_utils import run_bass_kernel_spmd

    nc = _get_nc()
    in_maps = _make_in_maps(hidden_states, attention_mask, q_w, q_b, kl_w, kl_b,
                            vl_w, vl_b, kr_w, kr_b, vr_w, vr_b, o_w)
    res = run_bass_kernel_spmd(nc, in_maps, core_ids=list(range(NCORES)))
    return _gather(res.results, o_b)
